# revision 16
# baseline (speedup 1.0000x reference)
"""Trainium2 Bass kernel for nn_MultiHeadAttention_67044439491211.

Mathematical note: the reference einsum 'bqkh,bvha->bqha' sums k and v
independently, so attn = (sum_k softmax(...)) * (sum_v v) = sum_v v
(softmax sums to 1 over k).  The whole module therefore collapses to

    out[b, q, :] = (sum_c context[b, c, :]) @ Wkv[:, D:] @ Wout

independent of q, query, Wq and mask.  The device kernel computes the
context reduction and the (folded) weight matmul, then broadcasts the
row across the q dimension and writes the full output shard.

Sharding: core c handles batch b = c//2 and output rows
[(c%2)*1024, (c%2+1)*1024).  Each core reads the full context of its
batch (needed for the complete reduction), so context is read twice
across the 8 cores.
"""

import numpy as np
import ml_dtypes

from concourse import bacc
import concourse.mybir as mybir
from concourse.tile import TileContext
from concourse.bass_utils import run_bass_kernel_spmd

B, QL, CL, D, H = 4, 2048, 2048, 512, 8
N_CORES = 8
ROWS_PER_CORE = QL // 2  # 1024

F32 = mybir.dt.float32
F32R = mybir.dt.float32r
BF16 = mybir.dt.bfloat16

# "bf16split": o = csum @ W2 via bf16 hi/lo decomposition (3 matmul passes,
#              ~1e-5 end-to-end error)
# "fp32r":     single-pass relaxed-precision fp32 matmuls (1 cycle/row)
O_MATMUL_MODE = "bf16split"

_NC_CACHE = {}


def _build_nc():
    nc = bacc.Bacc("TRN2", target_bir_lowering=False)

    ctx_h = nc.dram_tensor("ctx", [CL, D], F32, kind="ExternalInput")
    # host passes W2 (hi/lo) already in SBUF layout: [p, c*512+n] = W2[c*128+p, n]
    if O_MATMUL_MODE == "bf16split":
        w2hi_h = nc.dram_tensor("w2hi", [128, 4 * D], BF16, kind="ExternalInput")
        w2lo_h = nc.dram_tensor("w2lo", [128, 4 * D], BF16, kind="ExternalInput")
    else:
        w2_h = nc.dram_tensor("w2", [128, 4 * D], F32R, kind="ExternalInput")
    out_h = nc.dram_tensor("out", [ROWS_PER_CORE, D], F32, kind="ExternalOutput")

    P = 128
    G = 8            # context DMA groups (512 KB each)
    NT = 2           # consecutive rows per partition (G*P*NT == CL)
    DC = D // P      # 4 column chunks of 128

    # DRAM view: row = g*(P*NT) + p*NT + n -> partition p reads NT
    # consecutive rows (8KB contiguous) per group, one descriptor each
    ctx_v = ctx_h[:, :].rearrange("(g p n) d -> g p (n d)", g=G, p=P, n=NT)
    out_v = out_h[:, :].rearrange("(r p) n -> r p n", p=P)

    with TileContext(nc) as tc:
        with (
            tc.tile_pool(name="ctxp", bufs=8) as ctxp,
            tc.tile_pool(name="work", bufs=1) as work,
            tc.tile_pool(name="psum", bufs=1, space="PSUM") as psum,
        ):
            # context load first (the adds are the long pole); issue all on
            # the sync HWDGE ring (scalar ring has ~4us first-byte latency)
            tiles = []
            for g in range(G):
                t = ctxp.tile([P, NT * D], F32, tag="ctx")
                nc.sync.dma_start(out=t[:], in_=ctx_v[g])
                tiles.append(t)

            # weights on the scalar ring: issues in parallel with ctx and
            # the data isn't needed until the o-matmuls
            if O_MATMUL_MODE == "bf16split":
                w2hi_sb = work.tile([P, DC * D], BF16, tag="w2hi_sb")
                w2lo_sb = work.tile([P, DC * D], BF16, tag="w2lo_sb")
                nc.scalar.dma_start(out=w2hi_sb[:], in_=w2hi_h[:, :])
                nc.scalar.dma_start(out=w2lo_sb[:], in_=w2lo_h[:, :])
            else:
                w2_sb = work.tile([P, DC * D], F32R, tag="w2_sb")
                nc.scalar.dma_start(out=w2_sb[:], in_=w2_h[:, :])

            # constants
            ones = work.tile([P, 1], F32, tag="ones")
            nc.vector.memset(ones[:], 1.0)

            # accumulate every 512-column chunk straight into acc2 as each
            # tile lands; keeps the post-DMA tail to ~1.4us
            acc2 = work.tile([P, D], F32, tag="acc2")
            first = True
            for g in range(G):
                for k in range(NT):
                    chunk = tiles[g][:, k * D : (k + 1) * D]
                    if first:
                        nc.vector.tensor_copy(out=acc2[:], in_=chunk)
                        first = False
                    else:
                        nc.vector.tensor_add(out=acc2[:], in0=acc2[:], in1=chunk)

            # partition reduction via PE:  csumT[m, c] = sum_p acc2[p, c*128+m]
            csumT_ps = psum.tile([P, DC], F32, tag="csumT_ps")
            for c in range(DC):
                nc.tensor.matmul(
                    csumT_ps[:, c : c + 1],
                    acc2[:, c * P : (c + 1) * P],
                    ones[:],
                    start=True,
                    stop=True,
                )
            o_ps = psum.tile([1, D], F32, tag="o_ps")
            if O_MATMUL_MODE == "bf16split":
                csumT = work.tile([P, DC], F32, tag="csumT")
                nc.vector.tensor_copy(out=csumT[:], in_=csumT_ps[:])

                # split csumT into bf16 hi + lo for full-rate PE matmuls
                cs_hi = work.tile([P, DC], BF16, tag="cs_hi")
                cs_hi32 = work.tile([P, DC], F32, tag="cs_hi32")
                cs_lo32 = work.tile([P, DC], F32, tag="cs_lo32")
                cs_lo = work.tile([P, DC], BF16, tag="cs_lo")
                nc.vector.tensor_copy(out=cs_hi[:], in_=csumT[:])
                nc.vector.tensor_copy(out=cs_hi32[:], in_=cs_hi[:])
                nc.vector.tensor_sub(out=cs_lo32[:], in0=csumT[:], in1=cs_hi32[:])
                nc.vector.tensor_copy(out=cs_lo[:], in_=cs_lo32[:])

                # o[n] = sum_d csum[d] * W2[d, n]  (hi*hi + hi*lo + lo*hi)
                n_mm = 3 * DC
                i = 0
                for lhs_sb, rhs_sb in (
                    (cs_hi, w2hi_sb),
                    (cs_hi, w2lo_sb),
                    (cs_lo, w2hi_sb),
                ):
                    for c in range(DC):
                        nc.tensor.matmul(
                            o_ps[:],
                            lhs_sb[:, c : c + 1],
                            rhs_sb[:, c * D : (c + 1) * D],
                            start=(i == 0),
                            stop=(i == n_mm - 1),
                        )
                        i += 1
            else:
                # single-pass relaxed fp32 matmuls (1 cycle/row at N=512)
                csumT = work.tile([P, DC], F32R, tag="csumT")
                nc.vector.tensor_copy(out=csumT[:], in_=csumT_ps[:])
                for c in range(DC):
                    nc.tensor.matmul(
                        o_ps[:],
                        csumT[:, c : c + 1],
                        w2_sb[:, c * D : (c + 1) * D],
                        start=(c == 0),
                        stop=(c == DC - 1),
                    )

            # broadcast o to all 128 partitions, then write all row blocks
            o_sb = work.tile([1, D], F32, tag="o_sb")
            nc.vector.tensor_copy(out=o_sb[:], in_=o_ps[:])
            bcast = work.tile([P, D], F32, tag="bcast")
            nc.gpsimd.partition_broadcast(bcast[:], o_sb[:])

            # split the output across both HWDGE rings
            for r in range(ROWS_PER_CORE // P):
                eng = nc.sync if r % 2 == 0 else nc.scalar
                eng.dma_start(out=out_v[r], in_=bcast[:])

    nc.compile()
    return nc


def kernel(query=None, context=None, mask=None, Wq=None, Wkv=None, Wout=None,
           trace=False, **_ignored):
    context = np.asarray(context, dtype=np.float32)
    Wkv = np.asarray(Wkv, dtype=np.float32)
    Wout = np.asarray(Wout, dtype=np.float32)

    # fold the V projection and output projection into one matrix
    W2 = (Wkv[:, D:].astype(np.float64) @ Wout.astype(np.float64)).astype(np.float32)
    # pre-layout to SBUF shape: [p, c*512+n] = W2[c*128+p, n]
    W2sb = np.ascontiguousarray(
        W2.reshape(4, 128, D).transpose(1, 0, 2).reshape(128, 4 * D)
    )
    if O_MATMUL_MODE == "bf16split":
        w2hi = W2sb.astype(ml_dtypes.bfloat16)
        w2lo = (W2sb - w2hi.astype(np.float32)).astype(ml_dtypes.bfloat16)
        w_map = {"w2hi": w2hi, "w2lo": w2lo}
    else:
        w_map = {"w2": W2sb}

    if "nc" not in _NC_CACHE:
        _NC_CACHE["nc"] = _build_nc()
    nc = _NC_CACHE["nc"]

    in_maps = []
    for c in range(N_CORES):
        b = c // 2
        in_maps.append({"ctx": np.ascontiguousarray(context[b]), **w_map})

    res = run_bass_kernel_spmd(nc, in_maps, core_ids=list(range(N_CORES)),
                               trace=trace)
    kernel.last_results = res

    out = np.empty((B, QL, D), dtype=np.float32)
    for c in range(N_CORES):
        b, h = c // 2, c % 2
        out[b, h * ROWS_PER_CORE : (h + 1) * ROWS_PER_CORE, :] = res.results[c]["out"]
    return out


kernel.last_results = None


# revision 21
# speedup vs baseline: 1.0577x; 1.0577x over previous
"""Trainium2 Bass kernel for nn_MultiHeadAttention_67044439491211.

Mathematical note: the reference einsum 'bqkh,bvha->bqha' sums k and v
independently, so attn = (sum_k softmax(...)) * (sum_v v) = sum_v v
(softmax sums to 1 over k).  The whole module therefore collapses to

    out[b, q, :] = (sum_c context[b, c, :]) @ Wkv[:, D:] @ Wout

independent of q, query, Wq and mask.  The device kernel computes the
context reduction and the (folded) weight matmul, then broadcasts the
row across the q dimension and writes the full output shard.

Sharding: core c handles batch b = c//2 and output rows
[(c%2)*1024, (c%2+1)*1024).  Each core reads the full context of its
batch (needed for the complete reduction), so context is read twice
across the 8 cores.
"""

import numpy as np
import ml_dtypes

from concourse import bacc
import concourse.mybir as mybir
from concourse.tile import TileContext
from concourse.bass_utils import run_bass_kernel_spmd

B, QL, CL, D, H = 4, 2048, 2048, 512, 8
N_CORES = 8
ROWS_PER_CORE = QL // 2  # 1024

F32 = mybir.dt.float32
F32R = mybir.dt.float32r
BF16 = mybir.dt.bfloat16

# "bf16split": o = csum @ W2 via bf16 hi/lo decomposition (3 matmul passes,
#              ~1e-5 end-to-end error)
# "fp32r":     single-pass relaxed-precision fp32 matmuls (1 cycle/row)
O_MATMUL_MODE = "fp32r"

_NC_CACHE = {}


def _build_nc():
    nc = bacc.Bacc("TRN2", target_bir_lowering=False)

    ctx_h = nc.dram_tensor("ctx", [CL, D], F32, kind="ExternalInput")
    # host passes W2 (hi/lo) already in SBUF layout: [p, c*512+n] = W2[c*128+p, n]
    if O_MATMUL_MODE == "bf16split":
        w2hi_h = nc.dram_tensor("w2hi", [128, 4 * D], BF16, kind="ExternalInput")
        w2lo_h = nc.dram_tensor("w2lo", [128, 4 * D], BF16, kind="ExternalInput")
    else:
        w2_h = nc.dram_tensor("w2", [128, 4 * D], F32R, kind="ExternalInput")
    out_h = nc.dram_tensor("out", [ROWS_PER_CORE, D], F32, kind="ExternalOutput")

    P = 128
    G = 4            # context DMA groups (1 MB each)
    NT = 4           # consecutive rows per partition (G*P*NT == CL); the
                     # per-partition contiguous run (= DMA descriptor) is NT*2KB
    DC = D // P      # 4 column chunks of 128

    # DRAM view: row = g*(P*NT) + p*NT + n -> partition p reads NT
    # consecutive rows (8KB contiguous) per group, one descriptor each
    ctx_v = ctx_h[:, :].rearrange("(g p n) d -> g p (n d)", g=G, p=P, n=NT)
    out_v = out_h[:, :].rearrange("(r p) n -> r p n", p=P)

    with TileContext(nc) as tc:
        with (
            tc.tile_pool(name="ctxp", bufs=4) as ctxp,
            tc.tile_pool(name="work", bufs=1) as work,
            tc.tile_pool(name="psum", bufs=1, space="PSUM") as psum,
        ):
            # context load first (the adds are the long pole); issue all on
            # the sync HWDGE ring (scalar ring has ~4us first-byte latency)
            tiles = []
            for g in range(G):
                t = ctxp.tile([P, NT * D], F32, tag="ctx")
                nc.sync.dma_start(out=t[:], in_=ctx_v[g])
                tiles.append(t)

            # weights on the scalar ring: issues in parallel with ctx and
            # the data isn't needed until the o-matmuls
            if O_MATMUL_MODE == "bf16split":
                w2hi_sb = work.tile([P, DC * D], BF16, tag="w2hi_sb")
                w2lo_sb = work.tile([P, DC * D], BF16, tag="w2lo_sb")
                nc.scalar.dma_start(out=w2hi_sb[:], in_=w2hi_h[:, :])
                nc.scalar.dma_start(out=w2lo_sb[:], in_=w2lo_h[:, :])
            else:
                w2_sb = work.tile([P, DC * D], F32R, tag="w2_sb")
                nc.scalar.dma_start(out=w2_sb[:], in_=w2_h[:, :])

            # constants
            ones = work.tile([P, 1], F32, tag="ones")
            nc.vector.memset(ones[:], 1.0)

            # tiles 0+1: one wide add, folded to acc2 while tiles 2/3 are
            # still in flight; tiles 2 and 3 then accumulate per 512-column
            # chunk so the post-DMA tail is only ~2.8us
            acc = work.tile([P, NT * D], F32, tag="acc")
            nc.vector.tensor_add(out=acc[:], in0=tiles[0][:], in1=tiles[1][:])
            acc2 = work.tile([P, D], F32, tag="acc2")
            nc.vector.tensor_add(out=acc2[:], in0=acc[:, 0:D], in1=acc[:, D : 2 * D])
            nc.vector.tensor_add(out=acc2[:], in0=acc2[:], in1=acc[:, 2 * D : 3 * D])
            nc.vector.tensor_add(out=acc2[:], in0=acc2[:], in1=acc[:, 3 * D : 4 * D])
            for g in (2, 3):
                for k in range(NT):
                    nc.vector.tensor_add(
                        out=acc2[:],
                        in0=acc2[:],
                        in1=tiles[g][:, k * D : (k + 1) * D],
                    )

            # partition reduction via PE:  csumT[m, c] = sum_p acc2[p, c*128+m]
            csumT_ps = psum.tile([P, DC], F32, tag="csumT_ps")
            for c in range(DC):
                nc.tensor.matmul(
                    csumT_ps[:, c : c + 1],
                    acc2[:, c * P : (c + 1) * P],
                    ones[:],
                    start=True,
                    stop=True,
                )
            o_ps = psum.tile([1, D], F32, tag="o_ps")
            if O_MATMUL_MODE == "bf16split":
                csumT = work.tile([P, DC], F32, tag="csumT")
                nc.vector.tensor_copy(out=csumT[:], in_=csumT_ps[:])

                # split csumT into bf16 hi + lo for full-rate PE matmuls
                cs_hi = work.tile([P, DC], BF16, tag="cs_hi")
                cs_hi32 = work.tile([P, DC], F32, tag="cs_hi32")
                cs_lo32 = work.tile([P, DC], F32, tag="cs_lo32")
                cs_lo = work.tile([P, DC], BF16, tag="cs_lo")
                nc.vector.tensor_copy(out=cs_hi[:], in_=csumT[:])
                nc.vector.tensor_copy(out=cs_hi32[:], in_=cs_hi[:])
                nc.vector.tensor_sub(out=cs_lo32[:], in0=csumT[:], in1=cs_hi32[:])
                nc.vector.tensor_copy(out=cs_lo[:], in_=cs_lo32[:])

                # o[n] = sum_d csum[d] * W2[d, n]  (hi*hi + hi*lo + lo*hi)
                n_mm = 3 * DC
                i = 0
                for lhs_sb, rhs_sb in (
                    (cs_hi, w2hi_sb),
                    (cs_hi, w2lo_sb),
                    (cs_lo, w2hi_sb),
                ):
                    for c in range(DC):
                        nc.tensor.matmul(
                            o_ps[:],
                            lhs_sb[:, c : c + 1],
                            rhs_sb[:, c * D : (c + 1) * D],
                            start=(i == 0),
                            stop=(i == n_mm - 1),
                        )
                        i += 1
            else:
                # single-pass relaxed fp32 matmuls (1 cycle/row at N=512)
                csumT = work.tile([P, DC], F32R, tag="csumT")
                nc.vector.tensor_copy(out=csumT[:], in_=csumT_ps[:])
                for c in range(DC):
                    nc.tensor.matmul(
                        o_ps[:],
                        csumT[:, c : c + 1],
                        w2_sb[:, c * D : (c + 1) * D],
                        start=(c == 0),
                        stop=(c == DC - 1),
                    )

            # broadcast o to all 128 partitions, then write all row blocks
            o_sb = work.tile([1, D], F32, tag="o_sb")
            nc.vector.tensor_copy(out=o_sb[:], in_=o_ps[:])
            bcast = work.tile([P, D], F32, tag="bcast")
            nc.gpsimd.partition_broadcast(bcast[:], o_sb[:])

            for r in range(ROWS_PER_CORE // P):
                nc.sync.dma_start(out=out_v[r], in_=bcast[:])

    nc.compile()
    return nc


def kernel(query=None, context=None, mask=None, Wq=None, Wkv=None, Wout=None,
           trace=False, **_ignored):
    context = np.asarray(context, dtype=np.float32)
    Wkv = np.asarray(Wkv, dtype=np.float32)
    Wout = np.asarray(Wout, dtype=np.float32)

    # fold the V projection and output projection into one matrix
    W2 = (Wkv[:, D:].astype(np.float64) @ Wout.astype(np.float64)).astype(np.float32)
    # pre-layout to SBUF shape: [p, c*512+n] = W2[c*128+p, n]
    W2sb = np.ascontiguousarray(
        W2.reshape(4, 128, D).transpose(1, 0, 2).reshape(128, 4 * D)
    )
    if O_MATMUL_MODE == "bf16split":
        w2hi = W2sb.astype(ml_dtypes.bfloat16)
        w2lo = (W2sb - w2hi.astype(np.float32)).astype(ml_dtypes.bfloat16)
        w_map = {"w2hi": w2hi, "w2lo": w2lo}
    else:
        w_map = {"w2": W2sb}

    if "nc" not in _NC_CACHE:
        _NC_CACHE["nc"] = _build_nc()
    nc = _NC_CACHE["nc"]

    in_maps = []
    for c in range(N_CORES):
        b = c // 2
        in_maps.append({"ctx": np.ascontiguousarray(context[b]), **w_map})

    res = run_bass_kernel_spmd(nc, in_maps, core_ids=list(range(N_CORES)),
                               trace=trace)
    kernel.last_results = res

    out = np.empty((B, QL, D), dtype=np.float32)
    for c in range(N_CORES):
        b, h = c // 2, c % 2
        out[b, h * ROWS_PER_CORE : (h + 1) * ROWS_PER_CORE, :] = res.results[c]["out"]
    return out


kernel.last_results = None


# revision 24
# speedup vs baseline: 1.1185x; 1.0575x over previous
"""Trainium2 Bass kernel for nn_MultiHeadAttention_67044439491211.

Mathematical note: the reference einsum 'bqkh,bvha->bqha' sums k and v
independently, so attn = (sum_k softmax(...)) * (sum_v v) = sum_v v
(softmax sums to 1 over k).  The whole module therefore collapses to

    out[b, q, :] = (sum_c context[b, c, :]) @ Wkv[:, D:] @ Wout

independent of q, query, Wq and mask.  The device kernel computes the
context reduction and the (folded) weight matmul, then broadcasts the
row across the q dimension and writes the full output shard.

Sharding: core c handles batch b = c//2 and output rows
[(c%2)*1024, (c%2+1)*1024).  Each core reads the full context of its
batch (needed for the complete reduction), so context is read twice
across the 8 cores.
"""

import numpy as np
import ml_dtypes

from concourse import bacc
import concourse.mybir as mybir
from concourse.tile import TileContext
from concourse.bass_utils import run_bass_kernel_spmd

B, QL, CL, D, H = 4, 2048, 2048, 512, 8
N_CORES = 8
ROWS_PER_CORE = QL // 2  # 1024

F32 = mybir.dt.float32
F32R = mybir.dt.float32r
BF16 = mybir.dt.bfloat16

# "bf16split": o = csum @ W2 via bf16 hi/lo decomposition (3 matmul passes,
#              ~1e-5 end-to-end error)
# "fp32r":     single-pass relaxed-precision fp32 matmuls (1 cycle/row)
O_MATMUL_MODE = "fp32r"

_NC_CACHE = {}


def _build_nc():
    nc = bacc.Bacc("TRN2", target_bir_lowering=False, enable_partition_id=False)

    ctx_h = nc.dram_tensor("ctx", [CL, D], F32, kind="ExternalInput")
    # host passes W2 (hi/lo) already in SBUF layout: [p, c*512+n] = W2[c*128+p, n]
    if O_MATMUL_MODE == "bf16split":
        w2hi_h = nc.dram_tensor("w2hi", [128, 4 * D], BF16, kind="ExternalInput")
        w2lo_h = nc.dram_tensor("w2lo", [128, 4 * D], BF16, kind="ExternalInput")
    else:
        w2_h = nc.dram_tensor("w2", [128, 4 * D], F32R, kind="ExternalInput")
    out_h = nc.dram_tensor("out", [ROWS_PER_CORE, D], F32, kind="ExternalOutput")

    P = 128
    G = 4            # context DMA groups (1 MB each)
    NT = 4           # consecutive rows per partition (G*P*NT == CL); the
                     # per-partition contiguous run (= DMA descriptor) is NT*2KB
    DC = D // P      # 4 column chunks of 128

    # DRAM view: row = g*(P*NT) + p*NT + n -> partition p reads NT
    # consecutive rows (8KB contiguous) per group, one descriptor each
    ctx_v = ctx_h[:, :].rearrange("(g p n) d -> g p (n d)", g=G, p=P, n=NT)
    out_v = out_h[:, :].rearrange("(r p) n -> r p n", p=P)

    with TileContext(nc) as tc:
        with (
            tc.tile_pool(name="ctxp", bufs=4) as ctxp,
            tc.tile_pool(name="work", bufs=1) as work,
            tc.tile_pool(name="psum", bufs=1, space="PSUM") as psum,
        ):
            # context load first (the adds are the long pole); issue all on
            # the sync HWDGE ring (scalar ring has ~4us first-byte latency)
            tiles = []
            for g in range(G):
                t = ctxp.tile([P, NT * D], F32, tag="ctx")
                nc.sync.dma_start(out=t[:], in_=ctx_v[g])
                tiles.append(t)

            # weights queue on the same sync ring BEHIND ctx: they drain in
            # the idle window after ctx with no packet-slot contention
            # (putting them on the scalar ring delays ctx by ~4us)
            if O_MATMUL_MODE == "bf16split":
                w2hi_sb = work.tile([P, DC * D], BF16, tag="w2hi_sb")
                w2lo_sb = work.tile([P, DC * D], BF16, tag="w2lo_sb")
                nc.sync.dma_start(out=w2hi_sb[:], in_=w2hi_h[:, :])
                nc.sync.dma_start(out=w2lo_sb[:], in_=w2lo_h[:, :])
            else:
                w2_sb = work.tile([P, DC * D], F32R, tag="w2_sb")
                nc.sync.dma_start(out=w2_sb[:], in_=w2_h[:, :])

            # constants
            ones = work.tile([P, 1], F32, tag="ones")
            nc.vector.memset(ones[:], 1.0)

            # wide adds are the most DVE-efficient form (1.15us/MB vs
            # 1.38us/MB for 512-col chunks)
            acc = work.tile([P, NT * D], F32, tag="acc")
            nc.vector.tensor_add(out=acc[:], in0=tiles[0][:], in1=tiles[1][:])
            nc.vector.tensor_add(out=acc[:], in0=acc[:], in1=tiles[2][:])
            nc.vector.tensor_add(out=acc[:], in0=acc[:], in1=tiles[3][:])

            # fold the NT row-tiles:  acc2[p, d] = sum_n acc[p, n*D+d]
            acc2 = work.tile([P, D], F32, tag="acc2")
            nc.vector.tensor_add(out=acc2[:], in0=acc[:, 0:D], in1=acc[:, D : 2 * D])
            nc.vector.tensor_add(out=acc2[:], in0=acc2[:], in1=acc[:, 2 * D : 3 * D])
            nc.vector.tensor_add(out=acc2[:], in0=acc2[:], in1=acc[:, 3 * D : 4 * D])

            # partition reduction via PE:  csumT[m, c] = sum_p acc2[p, c*128+m]
            csumT_ps = psum.tile([P, DC], F32, tag="csumT_ps")
            for c in range(DC):
                nc.tensor.matmul(
                    csumT_ps[:, c : c + 1],
                    acc2[:, c * P : (c + 1) * P],
                    ones[:],
                    start=True,
                    stop=True,
                )
            o_ps = psum.tile([1, D], F32, tag="o_ps")
            if O_MATMUL_MODE == "bf16split":
                csumT = work.tile([P, DC], F32, tag="csumT")
                nc.vector.tensor_copy(out=csumT[:], in_=csumT_ps[:])

                # split csumT into bf16 hi + lo for full-rate PE matmuls
                cs_hi = work.tile([P, DC], BF16, tag="cs_hi")
                cs_hi32 = work.tile([P, DC], F32, tag="cs_hi32")
                cs_lo32 = work.tile([P, DC], F32, tag="cs_lo32")
                cs_lo = work.tile([P, DC], BF16, tag="cs_lo")
                nc.vector.tensor_copy(out=cs_hi[:], in_=csumT[:])
                nc.vector.tensor_copy(out=cs_hi32[:], in_=cs_hi[:])
                nc.vector.tensor_sub(out=cs_lo32[:], in0=csumT[:], in1=cs_hi32[:])
                nc.vector.tensor_copy(out=cs_lo[:], in_=cs_lo32[:])

                # o[n] = sum_d csum[d] * W2[d, n]  (hi*hi + hi*lo + lo*hi)
                n_mm = 3 * DC
                i = 0
                for lhs_sb, rhs_sb in (
                    (cs_hi, w2hi_sb),
                    (cs_hi, w2lo_sb),
                    (cs_lo, w2hi_sb),
                ):
                    for c in range(DC):
                        nc.tensor.matmul(
                            o_ps[:],
                            lhs_sb[:, c : c + 1],
                            rhs_sb[:, c * D : (c + 1) * D],
                            start=(i == 0),
                            stop=(i == n_mm - 1),
                        )
                        i += 1
            else:
                # single-pass relaxed fp32 matmuls (1 cycle/row at N=512)
                csumT = work.tile([P, DC], F32R, tag="csumT")
                nc.vector.tensor_copy(out=csumT[:], in_=csumT_ps[:])
                for c in range(DC):
                    nc.tensor.matmul(
                        o_ps[:],
                        csumT[:, c : c + 1],
                        w2_sb[:, c * D : (c + 1) * D],
                        start=(c == 0),
                        stop=(c == DC - 1),
                    )

            # broadcast o to all 128 partitions, then write all row blocks
            o_sb = work.tile([1, D], F32, tag="o_sb")
            nc.vector.tensor_copy(out=o_sb[:], in_=o_ps[:])
            bcast = work.tile([P, D], F32, tag="bcast")
            nc.gpsimd.partition_broadcast(bcast[:], o_sb[:])

            for r in range(ROWS_PER_CORE // P):
                nc.sync.dma_start(out=out_v[r], in_=bcast[:])

    nc.compile()
    return nc


def kernel(query=None, context=None, mask=None, Wq=None, Wkv=None, Wout=None,
           trace=False, **_ignored):
    context = np.asarray(context, dtype=np.float32)
    Wkv = np.asarray(Wkv, dtype=np.float32)
    Wout = np.asarray(Wout, dtype=np.float32)

    # fold the V projection and output projection into one matrix
    W2 = (Wkv[:, D:].astype(np.float64) @ Wout.astype(np.float64)).astype(np.float32)
    # pre-layout to SBUF shape: [p, c*512+n] = W2[c*128+p, n]
    W2sb = np.ascontiguousarray(
        W2.reshape(4, 128, D).transpose(1, 0, 2).reshape(128, 4 * D)
    )
    if O_MATMUL_MODE == "bf16split":
        w2hi = W2sb.astype(ml_dtypes.bfloat16)
        w2lo = (W2sb - w2hi.astype(np.float32)).astype(ml_dtypes.bfloat16)
        w_map = {"w2hi": w2hi, "w2lo": w2lo}
    else:
        w_map = {"w2": W2sb}

    if "nc" not in _NC_CACHE:
        _NC_CACHE["nc"] = _build_nc()
    nc = _NC_CACHE["nc"]

    in_maps = []
    for c in range(N_CORES):
        b = c // 2
        in_maps.append({"ctx": np.ascontiguousarray(context[b]), **w_map})

    res = run_bass_kernel_spmd(nc, in_maps, core_ids=list(range(N_CORES)),
                               trace=trace)
    kernel.last_results = res

    out = np.empty((B, QL, D), dtype=np.float32)
    for c in range(N_CORES):
        b, h = c // 2, c % 2
        out[b, h * ROWS_PER_CORE : (h + 1) * ROWS_PER_CORE, :] = res.results[c]["out"]
    return out


kernel.last_results = None


# revision 25
# speedup vs baseline: 1.2636x; 1.1297x over previous
"""Trainium2 Bass kernel for nn_MultiHeadAttention_67044439491211.

Mathematical note: the reference einsum 'bqkh,bvha->bqha' sums k and v
independently, so attn = (sum_k softmax(...)) * (sum_v v) = sum_v v
(softmax sums to 1 over k).  The whole module therefore collapses to

    out[b, q, :] = (sum_c context[b, c, :]) @ Wkv[:, D:] @ Wout

independent of q, query, Wq and mask.  The device kernel computes the
context reduction and the (folded) weight matmul, then broadcasts the
row across the q dimension and writes the full output shard.

Sharding: core c handles batch b = c//2 and output rows
[(c%2)*1024, (c%2+1)*1024).  Each core reads the full context of its
batch (needed for the complete reduction), so context is read twice
across the 8 cores.
"""

import numpy as np
import ml_dtypes

from concourse import bacc
import concourse.mybir as mybir
from concourse.tile import TileContext
from concourse.bass_utils import run_bass_kernel_spmd

B, QL, CL, D, H = 4, 2048, 2048, 512, 8
N_CORES = 8
ROWS_PER_CORE = QL // 2  # 1024

F32 = mybir.dt.float32
F32R = mybir.dt.float32r
BF16 = mybir.dt.bfloat16

# "bf16split": o = csum @ W2 via bf16 hi/lo decomposition (3 matmul passes,
#              ~1e-5 end-to-end error)
# "fp32r":     single-pass relaxed-precision fp32 matmuls (1 cycle/row)
O_MATMUL_MODE = "fp32r"

_NC_CACHE = {}


def _build_nc():
    nc = bacc.Bacc("TRN2", target_bir_lowering=False, enable_partition_id=False)

    ctx_h = nc.dram_tensor("ctx", [CL, D], F32, kind="ExternalInput")
    # host passes W2 (hi/lo) already in SBUF layout: [p, c*512+n] = W2[c*128+p, n]
    if O_MATMUL_MODE == "bf16split":
        w2hi_h = nc.dram_tensor("w2hi", [128, 4 * D], BF16, kind="ExternalInput")
        w2lo_h = nc.dram_tensor("w2lo", [128, 4 * D], BF16, kind="ExternalInput")
    else:
        w2_h = nc.dram_tensor("w2", [128, 4 * D], F32R, kind="ExternalInput")
    out_h = nc.dram_tensor("out", [ROWS_PER_CORE, D], F32, kind="ExternalOutput")

    P = 128
    G = 4            # context DMA groups (1 MB each)
    NT = 4           # consecutive rows per partition (G*P*NT == CL); the
                     # per-partition contiguous run (= DMA descriptor) is NT*2KB
    DC = D // P      # 4 column chunks of 128

    # DRAM view: row = g*(P*NT) + p*NT + n -> partition p reads NT
    # consecutive rows (8KB contiguous) per group, one descriptor each
    ctx_v = ctx_h[:, :].rearrange("(g p n) d -> g p (n d)", g=G, p=P, n=NT)
    out_v = out_h[:, :].rearrange("(r p) n -> r p n", p=P)

    with TileContext(nc) as tc:
        with (
            tc.tile_pool(name="ctxp", bufs=4) as ctxp,
            tc.tile_pool(name="work", bufs=1) as work,
            tc.tile_pool(name="psum", bufs=1, space="PSUM") as psum,
        ):
            # context load first (the adds are the long pole); issue all on
            # the sync HWDGE ring (scalar ring has ~4us first-byte latency)
            tiles = []
            for g in range(G):
                t = ctxp.tile([P, NT * D], F32, tag="ctx")
                nc.sync.dma_start(out=t[:], in_=ctx_v[g])
                tiles.append(t)

            # weights queue on the same sync ring BEHIND ctx: they drain in
            # the idle window after ctx with no packet-slot contention
            # (putting them on the scalar ring delays ctx by ~4us)
            if O_MATMUL_MODE == "bf16split":
                w2hi_sb = work.tile([P, DC * D], BF16, tag="w2hi_sb")
                w2lo_sb = work.tile([P, DC * D], BF16, tag="w2lo_sb")
                nc.sync.dma_start(out=w2hi_sb[:], in_=w2hi_h[:, :])
                nc.sync.dma_start(out=w2lo_sb[:], in_=w2lo_h[:, :])
            else:
                w2_sb = work.tile([P, DC * D], F32R, tag="w2_sb")
                nc.sync.dma_start(out=w2_sb[:], in_=w2_h[:, :])

            # constants
            ones = work.tile([P, 1], F32, tag="ones")
            nc.vector.memset(ones[:], 1.0)

            # wide adds are the most DVE-efficient form (1.15us/MB vs
            # 1.38us/MB for 512-col chunks)
            acc = work.tile([P, NT * D], F32, tag="acc")
            nc.vector.tensor_add(out=acc[:], in0=tiles[0][:], in1=tiles[1][:])
            nc.vector.tensor_add(out=acc[:], in0=acc[:], in1=tiles[2][:])
            nc.vector.tensor_add(out=acc[:], in0=acc[:], in1=tiles[3][:])

            # fold the NT row-tiles:  acc2[p, d] = sum_n acc[p, n*D+d]
            acc2 = work.tile([P, D], F32, tag="acc2")
            nc.vector.tensor_add(out=acc2[:], in0=acc[:, 0:D], in1=acc[:, D : 2 * D])
            nc.vector.tensor_add(out=acc2[:], in0=acc2[:], in1=acc[:, 2 * D : 3 * D])
            nc.vector.tensor_add(out=acc2[:], in0=acc2[:], in1=acc[:, 3 * D : 4 * D])

            # partition reduction via PE:  csumT[m, c] = sum_p acc2[p, c*128+m]
            csumT_ps = psum.tile([P, DC], F32, tag="csumT_ps")
            for c in range(DC):
                nc.tensor.matmul(
                    csumT_ps[:, c : c + 1],
                    acc2[:, c * P : (c + 1) * P],
                    ones[:],
                    start=True,
                    stop=True,
                )
            # o-matmuls with a column-broadcast stationary operand:
            # lhsT[k, m] = csumT[k, c] for every m, so every output row of
            # the (128, 512) PSUM tile is o[n] — the q-broadcast falls out
            # of the matmul for free.
            bc_ps = psum.tile([P, D], F32, tag="bc_ps")
            if O_MATMUL_MODE == "bf16split":
                csumT = work.tile([P, DC], F32, tag="csumT")
                nc.vector.tensor_copy(out=csumT[:], in_=csumT_ps[:])

                # split csumT into bf16 hi + lo for full-rate PE matmuls
                cs_hi = work.tile([P, DC], BF16, tag="cs_hi")
                cs_hi32 = work.tile([P, DC], F32, tag="cs_hi32")
                cs_lo32 = work.tile([P, DC], F32, tag="cs_lo32")
                cs_lo = work.tile([P, DC], BF16, tag="cs_lo")
                nc.vector.tensor_copy(out=cs_hi[:], in_=csumT[:])
                nc.vector.tensor_copy(out=cs_hi32[:], in_=cs_hi[:])
                nc.vector.tensor_sub(out=cs_lo32[:], in0=csumT[:], in1=cs_hi32[:])
                nc.vector.tensor_copy(out=cs_lo[:], in_=cs_lo32[:])

                # o[n] = sum_d csum[d] * W2[d, n]  (hi*hi + hi*lo + lo*hi)
                n_mm = 3 * DC
                i = 0
                for lhs_sb, rhs_sb in (
                    (cs_hi, w2hi_sb),
                    (cs_hi, w2lo_sb),
                    (cs_lo, w2hi_sb),
                ):
                    for c in range(DC):
                        nc.tensor.matmul(
                            bc_ps[:],
                            lhs_sb[:, c : c + 1].broadcast_to([P, P]),
                            rhs_sb[:, c * D : (c + 1) * D],
                            start=(i == 0),
                            stop=(i == n_mm - 1),
                        )
                        i += 1
            else:
                # single-pass relaxed fp32 matmuls (1 cycle/row at N=512)
                csumT = work.tile([P, DC], F32R, tag="csumT")
                nc.vector.tensor_copy(out=csumT[:], in_=csumT_ps[:])
                for c in range(DC):
                    nc.tensor.matmul(
                        bc_ps[:],
                        csumT[:, c : c + 1].broadcast_to([P, P]),
                        w2_sb[:, c * D : (c + 1) * D],
                        start=(c == 0),
                        stop=(c == DC - 1),
                    )

            bcast = work.tile([P, D], F32, tag="bcast")
            nc.vector.tensor_copy(out=bcast[:], in_=bc_ps[:])

            # alternate the HWDGE rings: by now the input is fully drained,
            # so the scalar ring is contention-free and the ~0.6us issue ops
            # pipeline two wide
            for r in range(ROWS_PER_CORE // P):
                eng = nc.sync if r % 2 == 0 else nc.scalar
                eng.dma_start(out=out_v[r], in_=bcast[:])

    nc.compile()
    return nc


def kernel(query=None, context=None, mask=None, Wq=None, Wkv=None, Wout=None,
           trace=False, **_ignored):
    context = np.asarray(context, dtype=np.float32)
    Wkv = np.asarray(Wkv, dtype=np.float32)
    Wout = np.asarray(Wout, dtype=np.float32)

    # fold the V projection and output projection into one matrix
    W2 = (Wkv[:, D:].astype(np.float64) @ Wout.astype(np.float64)).astype(np.float32)
    # pre-layout to SBUF shape: [p, c*512+n] = W2[c*128+p, n]
    W2sb = np.ascontiguousarray(
        W2.reshape(4, 128, D).transpose(1, 0, 2).reshape(128, 4 * D)
    )
    if O_MATMUL_MODE == "bf16split":
        w2hi = W2sb.astype(ml_dtypes.bfloat16)
        w2lo = (W2sb - w2hi.astype(np.float32)).astype(ml_dtypes.bfloat16)
        w_map = {"w2hi": w2hi, "w2lo": w2lo}
    else:
        w_map = {"w2": W2sb}

    if "nc" not in _NC_CACHE:
        _NC_CACHE["nc"] = _build_nc()
    nc = _NC_CACHE["nc"]

    in_maps = []
    for c in range(N_CORES):
        b = c // 2
        in_maps.append({"ctx": np.ascontiguousarray(context[b]), **w_map})

    res = run_bass_kernel_spmd(nc, in_maps, core_ids=list(range(N_CORES)),
                               trace=trace)
    kernel.last_results = res

    out = np.empty((B, QL, D), dtype=np.float32)
    for c in range(N_CORES):
        b, h = c // 2, c % 2
        out[b, h * ROWS_PER_CORE : (h + 1) * ROWS_PER_CORE, :] = res.results[c]["out"]
    return out


kernel.last_results = None
